# revision 1
# baseline (speedup 1.0000x reference)
"""Trainium2 Bass kernel for nn_BiLSTM pairwise-scores problem.

Math (reference):
  vec  = concat(word_emb[wi], pos_emb[pi], ext_emb[ei])          [512, 425]
  h    = concat(lstm_cell_f(vec), lstm_cell_b(vec))              [512, 200]
  cat  = [h, vec] for t <= 255 else [vec, h]                     [512, 625]
  f    = cat @ w_mlp_in.T + b_mlp_in                             [512, 400]
  out  = tanh((f[:,None,:] + f[None,:,:]) @ w_mlp_out.T + b_out) [512, 512, 42]

Key factorization: (f_i + f_j) @ W.T + b = g'_i + g'_j with
g' = f @ W.T + b/2, so the O(n^2 * 400 * 42) matmul collapses to a
[512, 42] projection plus a pairwise broadcast-add, implemented on the PE
as a single K=43 matmul per output chunk: lhsT = [g'_i rows; ones row],
rhs = [periodic identity rows; g'_j flattened row].

Sharding: 8 cores = 4 i-blocks (128 rows) x 2 j-halves (256 cols).
Each core runs an identical (SPMD) program on a permuted 384-token slice:
cols 0:128 = its i-block tokens, cols 128:384 = its j-half tokens.
The embedding gather and weight layout (transposes / gate stacking /
hv-vs-vh row ordering) happen on the host; all dense compute (LSTM cells,
MLPs, pairwise + tanh) runs on device in bf16 with fp32 PSUM accumulation.
"""

import os
import sys

import numpy as np

for _p in ("/opt/trn_rl_repo", "/root/.axon_site/_ro/trn_rl_repo"):
    if os.path.isdir(_p) and _p not in sys.path:
        sys.path.insert(0, _p)

import ml_dtypes  # noqa: E402

import concourse.bacc as bacc  # noqa: E402
import concourse.bass as bass  # noqa: E402
import concourse.mybir as mybir  # noqa: E402
from concourse.bass_utils import run_bass_kernel_spmd  # noqa: E402
from concourse.tile import TileContext  # noqa: E402

BF16 = mybir.dt.bfloat16
F32 = mybir.dt.float32
AF = mybir.ActivationFunctionType

SEQ = 512
D_VEC = 425  # 100 + 25 + 300
NREL = 42
T = 384  # per-core tokens: 128 (i-block) + 256 (j-half)
NFLAT = 256 * NREL  # 10752 = per-core output row length
N_CHUNK = 512
N_CHUNKS = NFLAT // N_CHUNK  # 21
GRP = 4  # pairwise chunks fused per PSUM group / tanh / DMA
IC_PER = 16 * NREL  # 672: replication period for the identity pattern

# K-dim tiling of the 425-dim feature axis
KS = [(0, 128), (128, 256), (256, 384), (384, 425)]
# gate order in the stacked [425, 600] gate weight: i_f g_f o_f i_b g_b o_b
GATE_FUNCS = [AF.Sigmoid, AF.Tanh, AF.Sigmoid] * 2
# PSUM slot split between the gate stream (pg) and mlp_in (pf): the gate
# stream needs 4 slots to stay dense past the first ACT evacuation (which
# keeps the PE clock-warmup lottery winnable); mlp_in tolerates 2.
PG_BUFS = 5 if os.environ.get("KV_PSUM2") == "E" else 4
PF_BUFS = 3 if os.environ.get("KV_PSUM2") == "E" else 2
PNAT_TAG = "pf" if os.environ.get("KV_PSUM2") == "E" else "pnat"

# ---- packed bf16 constant layout: [128, NPK] ----
_SEGS = []  # name -> (rows, col_off, width)


def _seg(name, rows, width):
    off = _SEGS[-1][2] + _SEGS[-1][3] if _SEGS else 0
    _SEGS.append((name, rows, off, width))


# interleaved (vt_k, g6_k) pairs so the first gate matmuls can start as
# soon as the first small DMA lands — the gate matmul stream itself then
# warms the PE clock (no dummy warmup needed).
for _k, (_a, _b) in enumerate(KS):
    _seg(f"vt{_k}", _b - _a, 384)
    _seg(f"g6{_k}", _b - _a, 600)
for _g in range(2):
    for _a2 in range(2):
        _seg(f"wh{_g}{_a2}", 100, 400)
for _g in range(2):
    for _k, (_a, _b) in enumerate(KS):
        _seg(f"wv{_g}{_k}", _b - _a, 400)
_seg("wo", 101, 4 * NREL)
_seg("ic", NREL, IC_PER)
SEG = {s[0]: s for s in _SEGS}
NPK = _SEGS[-1][2] + _SEGS[-1][3]
# input DMA split points: one per (vt_k, g6_k) pair, then the remainder
PK_CUTS = [SEG[f"g6{_k}"][2] + SEG[f"g6{_k}"][3] for _k in range(4)] + [NPK]


def _build_program():
    nc = bacc.Bacc()

    pk_d = nc.dram_tensor("pk", [128, NPK], BF16, kind="ExternalInput")
    bias_d = nc.dram_tensor("bias", [100, 11], F32, kind="ExternalInput")
    out_d = nc.dram_tensor("out", [128, NFLAT], F32, kind="ExternalOutput")

    with TileContext(nc) as tc:
        with (
            tc.tile_pool(name="const", bufs=1) as cp,
            tc.tile_pool(name="work", bufs=3) as wp,
            tc.tile_pool(name="outp", bufs=5) as op_,
        ):
            # -------- early on-chip init (no DMA deps) --------
            # bias DMA first on the scalar queue: its ~2us completion
            # receipt must not gate the first gate activation (which gates
            # the pg PSUM rotation and thus the gate matmul density).
            wsrc = cp.tile([128, N_CHUNK], BF16, tag="wsrc")
            nc.gpsimd.memset(wsrc, 0.0)
            # lhsT of the pairwise matmul: rows 0:42 = g'_i, row 42 = 1.0.
            # DVE partition base must be 32-aligned, so memset 32:43 and let
            # the later g' write overwrite rows 32:42.
            el = cp.tile([NREL + 1, 128], BF16, tag="el")
            nc.vector.memset(el[32 : NREL + 1, :], 1.0)
            # warmup activations absorb the ACT table-set load early
            warm2 = cp.tile([1, 8], F32, tag="warm2")
            nc.scalar.activation(out=warm2, in_=wsrc[0:1, 0:8], func=AF.Sigmoid)
            nc.scalar.activation(out=warm2, in_=wsrc[0:1, 0:8], func=AF.Tanh)

            # -------- input DMAs (paired packed chunks + bias) --------
            # The gate matmul stream doubles as the PE clock warmup, so the
            # (vt_k, g6_k) pairs are DMA'd individually to land ASAP.
            # bias rides the scalar engine's HWDGE queue so it neither
            # waits behind nor delays the pk stream on sync
            bias = cp.tile([100, 11], F32, tag="bias")
            nc.scalar.dma_start(out=bias, in_=bias_d[:, :])
            pk = cp.tile([128, NPK], BF16, tag="pk")
            prev = 0
            for cut in PK_CUTS[:4]:
                nc.sync.dma_start(out=pk[:, prev:cut], in_=pk_d[:, prev:cut])
                prev = cut
            nc.sync.dma_start(out=pk[:, prev:NPK], in_=pk_d[:, prev:NPK])

            def seg(name):
                _, rows, off, width = SEG[name]
                return pk[0:rows, off : off + width]

            vt = [seg(f"vt{k}") for k in range(4)]
            g6 = [seg(f"g6{k}") for k in range(4)]
            wh = [[seg(f"wh{g}{a}") for a in range(2)] for g in range(2)]
            wv = [[seg(f"wv{g}{k}") for k in range(4)] for g in range(2)]
            wo = seg("wo")
            ic = seg("ic")

            # pairwise rhs: rows 0:42 = periodic identity, row 42 = g'_j flat
            rr = cp.tile([NREL + 1, NFLAT], BF16, tag="rr")
            ic_rep = bass.AP(
                tensor=ic.tensor,
                offset=ic.offset,
                ap=[ic.ap[0], [0, NFLAT // IC_PER], ic.ap[1]],
            )
            nc.sync.dma_start(out=rr[0:NREL, :], in_=ic_rep)

            with tc.tile_pool(name="psum_pre", bufs=1, space="PSUM") as pp:
                # -------- LSTM gates (both dirs, f-gate skipped) --------
                # Per-direction ordering: i, g (then c = sig(i)*tanh(g) and
                # tanh(c) start immediately), then o, then h — shortens the
                # serial ACT chain to each direction's h.
                def gate(m):
                    pg = pp.tile([100, T], F32, tag="pg", bufs=PG_BUFS, name=f"pg{m}")
                    for k in range(4):
                        nc.tensor.matmul(
                            pg,
                            lhsT=g6[k][:, m * 100 : (m + 1) * 100],
                            rhs=vt[k],
                            start=(k == 0),
                            stop=(k == 3),
                        )
                    a_ = wp.tile([100, T], BF16, tag=f"act{m}", name=f"act{m}")
                    nc.scalar.activation(
                        out=a_,
                        in_=pg,
                        func=GATE_FUNCS[m],
                        bias=bias[0:100, m : m + 1],
                        scale=1.0,
                    )
                    return a_

                acts = [None] * 6
                hh = []
                for d in range(2):
                    si = gate(3 * d)
                    tg = gate(3 * d + 1)
                    c_ = wp.tile([100, T], BF16, tag=f"c{d}")
                    nc.vector.tensor_mul(c_, si, tg)
                    tc_ = wp.tile([100, T], BF16, tag=f"tc{d}")
                    nc.scalar.activation(out=tc_, in_=c_, func=AF.Tanh)
                    so = gate(3 * d + 2)
                    h_ = cp.tile([100, T], BF16, tag=f"h{d}")
                    nc.vector.tensor_mul(h_, so, tc_)
                    hh.append(h_)
                    acts[3 * d : 3 * d + 3] = [si, tg, so]
                # fillers pinned into the LSTM ACT/DVE gap: keep the PE
                # activity monitor from re-throttling the clock. Gated on
                # the last gate activation so the scheduler can't hoist
                # them ahead of the gap.
                pfill = pp.tile([100, T], F32, tag="pg", bufs=PG_BUFS, name="pfill")
                for _ in range(6):
                    nc.tensor.matmul(
                        pfill,
                        lhsT=acts[5][:, 0:100],
                        rhs=acts[5],
                        start=True,
                        stop=True,
                    )

                # -------- mlp_in: fT [400, 384] --------
                # vec pieces first (no h dependency), h pieces close the
                # accumulation group so the PE overlaps the LSTM ACT chain.
                groups = [(0, 0, 128), (1, 128, 384)]  # (g, col_a, col_b)
                fm = []
                for m in range(4):
                    ms = slice(m * 100, (m + 1) * 100)
                    pf = pp.tile([100, T], F32, tag="pf", bufs=PF_BUFS)
                    for g, ca, cb in groups:
                        for k in range(4):
                            nc.tensor.matmul(
                                pf[:, ca:cb],
                                lhsT=wv[g][k][:, ms],
                                rhs=vt[k][:, ca:cb],
                                start=(k == 0),
                                stop=False,
                            )
                        for a in range(2):
                            nc.tensor.matmul(
                                pf[:, ca:cb],
                                lhsT=wh[g][a][:, ms],
                                rhs=hh[a][:, ca:cb],
                                start=False,
                                stop=(a == 1),
                            )
                    # fm[3] carries an extra all-ones row 100 so the natural-
                    # layout mlp_out below can fold +b_out/2 in as a rank-1
                    # term (wo row 100 holds b_out/2). Memset base must be
                    # 32-aligned: set 96:101, rows 96:100 overwritten below.
                    rows = 101 if m == 3 else 100
                    f_ = cp.tile([rows, T], BF16, tag=f"f{m}")
                    if m == 3:
                        nc.vector.memset(f_[96:101, :], 1.0)
                    nc.vector.tensor_scalar_add(
                        f_[0:100, :], pf, bias[0:100, 6 + m : 7 + m]
                    )
                    fm.append(f_)

                # -------- mlp_out, i-block: g'T [42, 128] (+ b_out/2) ----
                pl = pp.tile([NREL, 128], F32, tag="pg", bufs=PG_BUFS, name="pl")
                for m in range(4):
                    nc.tensor.matmul(
                        pl,
                        lhsT=wo[0:100, m * NREL : (m + 1) * NREL],
                        rhs=fm[m][0:100, 0:128],
                        start=(m == 0),
                        stop=(m == 3),
                    )
                nc.vector.tensor_scalar_add(
                    el[0:NREL, :], pl, bias[0:NREL, 10:11]
                )

                # -------- mlp_out, j-half: g' in natural layout ----------
                # g'_nat[t, r] = sum_f fT[f, t] * WoutT[f, r] (+ ones * b/2)
                # — fT is already [f, t], so no transposes are needed; the
                # flatten DMA reads the natural-layout tile partition-major.
                for c in range(2):
                    krows = [100, 100, 100, 101]
                    png = pp.tile([128, NREL], F32, tag=PNAT_TAG, bufs=(PF_BUFS if PNAT_TAG == "pf" else 2), name=f"png{c}")
                    for m in range(4):
                        kr = krows[m]
                        nc.tensor.matmul(
                            png,
                            lhsT=fm[m][0:kr, 128 + c * 128 : 256 + c * 128],
                            rhs=wo[0:kr, m * NREL : (m + 1) * NREL],
                            start=(m == 0),
                            stop=(m == 3),
                        )
                    tj = wp.tile([128, NREL], BF16, tag="tj")
                    nc.vector.tensor_copy(tj, png)
                    nc.sync.dma_start(
                        out=rr[NREL : NREL + 1, c * 128 * NREL : (c + 1) * 128 * NREL],
                        in_=tj,
                    )
                # fillers pinned into the flatten latency gap
                pfill3 = pp.tile([NREL, N_CHUNK], F32, tag="pg", bufs=PG_BUFS, name="pfill3")
                for _ in range(6):
                    nc.tensor.matmul(
                        pfill3,
                        lhsT=tj[:, 0:NREL],
                        rhs=wsrc,
                        start=True,
                        stop=True,
                    )

            # -------- pairwise: tanh(g'_i + g'_j) --------
            # Group sizes: small first group lets the (bottleneck) ACT
            # tanh stream start early; small last group keeps the tail
            # DMA short. Total ACT overhead is identical to uniform 4s.
            grp_plan = (2, 4, 4, 4, 4, 3)
            with tc.tile_pool(name="psum_pair", bufs=2, space="PSUM") as pq:
                c = 0
                for nch in grp_plan:
                    ppair = pq.tile([128, GRP * N_CHUNK], F32, tag="ppair")
                    base = c * N_CHUNK
                    for q in range(nch):
                        nc.tensor.matmul(
                            ppair[:, q * N_CHUNK : (q + 1) * N_CHUNK],
                            lhsT=el,
                            rhs=rr[:, (c + q) * N_CHUNK : (c + q + 1) * N_CHUNK],
                            start=True,
                            stop=True,
                        )
                    ot = op_.tile([128, GRP * N_CHUNK], F32, tag="ot")
                    nc.scalar.activation(
                        out=ot[:, 0 : nch * N_CHUNK],
                        in_=ppair[:, 0 : nch * N_CHUNK],
                        func=AF.Tanh,
                    )
                    nc.sync.dma_start(
                        out=out_d[:, base : base + nch * N_CHUNK],
                        in_=ot[:, 0 : nch * N_CHUNK],
                    )
                    c += nch

    nc.finalize()
    return nc


def _host_prepare(inputs):
    """Gather embeddings + lay out weights; returns per-core in_maps."""
    bf = ml_dtypes.bfloat16
    wi = np.asarray(inputs["word_idx"]).astype(np.int64)
    pi = np.asarray(inputs["pos_idx"]).astype(np.int64)
    ei = np.asarray(inputs["ext_idx"]).astype(np.int64)
    we = np.asarray(inputs["word_emb"], np.float32)
    pe = np.asarray(inputs["pos_emb"], np.float32)
    xe = np.asarray(inputs["ext_emb"], np.float32)
    vec = np.concatenate([we[wi], pe[pi], xe[ei]], axis=-1)  # [512, 425] f32

    w_ih_f = np.asarray(inputs["w_ih_f"], np.float32)
    w_ih_b = np.asarray(inputs["w_ih_b"], np.float32)
    b_f = np.asarray(inputs["b_f"], np.float32)
    b_b = np.asarray(inputs["b_b"], np.float32)
    w_mlp_in = np.asarray(inputs["w_mlp_in"], np.float32)
    b_mlp_in = np.asarray(inputs["b_mlp_in"], np.float32)
    w_mlp_out = np.asarray(inputs["w_mlp_out"], np.float32)
    b_mlp_out = np.asarray(inputs["b_mlp_out"], np.float32)

    # stacked gate weights [425, 600]: i_f g_f o_f i_b g_b o_b (f unused)
    w6 = np.concatenate(
        [
            w_ih_f[0:100],
            w_ih_f[200:300],
            w_ih_f[300:400],
            w_ih_b[0:100],
            w_ih_b[200:300],
            w_ih_b[300:400],
        ],
        axis=0,
    ).T  # [425, 600]

    bias = np.zeros((100, 11), np.float32)
    for m, sl in enumerate(
        [b_f[0:100], b_f[200:300], b_f[300:400], b_b[0:100], b_b[200:300], b_b[300:400]]
    ):
        bias[:, m] = sl
    bias[:, 6:10] = b_mlp_in.reshape(4, 100).T
    bias[0:NREL, 10] = 0.5 * b_mlp_out

    # row 100: b_out/2 for the natural-layout mlp_out rank-1 bias fold
    # (only the m=3 block's slice is ever read at K=101)
    wo = np.zeros((101, 4 * NREL), np.float32)
    wout_t = w_mlp_out.T  # [400, 42]
    for m in range(4):
        wo[0:100, m * NREL : (m + 1) * NREL] = wout_t[m * 100 : (m + 1) * 100]
        wo[100, m * NREL : (m + 1) * NREL] = 0.5 * b_mlp_out

    # periodic identity block for the pairwise broadcast matmul
    ic = np.zeros((NREL, IC_PER), np.float32)
    cols = np.arange(IC_PER)
    ic[cols % NREL, cols] = 1.0

    def halves(hv):
        if hv:  # cat = [h, vec]
            whx = w_mlp_in[:, 0:200].T  # [200, 400] rows = h features
            wvx = w_mlp_in[:, 200:625].T  # [425, 400] rows = vec features
        else:  # cat = [vec, h]
            whx = w_mlp_in[:, 425:625].T
            wvx = w_mlp_in[:, 0:425].T
        return whx, wvx

    def fill(pk, name, arr):
        _, rows, off, width = SEG[name]
        assert arr.shape == (rows, width), (name, arr.shape, rows, width)
        pk[0:rows, off : off + width] = arr

    in_maps = []
    for core in range(8):
        ib, jh = core // 2, core % 2
        toks = np.concatenate(
            [np.arange(ib * 128, (ib + 1) * 128), np.arange(jh * 256, (jh + 1) * 256)]
        )
        vect = vec[toks].T  # [425, 384]
        g0h, g0v = halves(ib < 2)
        g1h, g1v = halves(jh == 0)

        pk = np.zeros((128, NPK), np.float32)
        for k, (a, b) in enumerate(KS):
            fill(pk, f"vt{k}", vect[a:b])
            fill(pk, f"g6{k}", w6[a:b])
        for g, (gh, gv) in enumerate([(g0h, g0v), (g1h, g1v)]):
            for a in range(2):
                fill(pk, f"wh{g}{a}", gh[a * 100 : (a + 1) * 100])
            for k, (a, b) in enumerate(KS):
                fill(pk, f"wv{g}{k}", gv[a:b])
        fill(pk, "wo", wo)
        fill(pk, "ic", ic)
        in_maps.append(dict(pk=pk.astype(bf), bias=bias))
    return in_maps


_CACHED_NC = None


def kernel(**inputs):
    global _CACHED_NC
    in_maps = _host_prepare(inputs)
    if _CACHED_NC is None:
        _CACHED_NC = _build_program()
    res = run_bass_kernel_spmd(_CACHED_NC, in_maps, list(range(8)))
    full = np.empty((SEQ, SEQ, NREL), np.float32)
    for core in range(8):
        ib, jh = core // 2, core % 2
        blk = res.results[core]["out"].reshape(128, 256, NREL)
        full[ib * 128 : (ib + 1) * 128, jh * 256 : (jh + 1) * 256, :] = blk
    return full


if __name__ == "__main__":
    rng = np.random.default_rng(0)
    demo = dict(
        word_idx=rng.integers(0, 50000, 512),
        pos_idx=rng.integers(0, 48, 512),
        ext_idx=rng.integers(0, 100000, 512),
        word_emb=rng.standard_normal((50000, 100), np.float32) * 0.05,
        pos_emb=rng.standard_normal((48, 25), np.float32) * 0.05,
        ext_emb=rng.standard_normal((100000, 300), np.float32) * 0.05,
        w_ih_f=rng.standard_normal((400, 425), np.float32) * 0.05,
        b_f=rng.standard_normal(400).astype(np.float32) * 0.05,
        w_ih_b=rng.standard_normal((400, 425), np.float32) * 0.05,
        b_b=rng.standard_normal(400).astype(np.float32) * 0.05,
        w_mlp_in=rng.standard_normal((400, 625), np.float32) * 0.05,
        b_mlp_in=rng.standard_normal(400).astype(np.float32) * 0.05,
        w_mlp_out=rng.standard_normal((42, 400), np.float32) * 0.05,
        b_mlp_out=rng.standard_normal(42).astype(np.float32) * 0.05,
    )
    out = kernel(**demo)
    print("out", out.shape, out.dtype, float(np.abs(out).max()))



# revision 6
# speedup vs baseline: 1.1556x; 1.1556x over previous
"""Trainium2 Bass kernel for nn_BiLSTM pairwise-scores problem.

Math (reference):
  vec  = concat(word_emb[wi], pos_emb[pi], ext_emb[ei])          [512, 425]
  h    = concat(lstm_cell_f(vec), lstm_cell_b(vec))              [512, 200]
  cat  = [h, vec] for t <= 255 else [vec, h]                     [512, 625]
  f    = cat @ w_mlp_in.T + b_mlp_in                             [512, 400]
  out  = tanh((f[:,None,:] + f[None,:,:]) @ w_mlp_out.T + b_out) [512, 512, 42]

Key factorization: (f_i + f_j) @ W.T + b = g'_i + g'_j with
g' = f @ W.T + b/2, so the O(n^2 * 400 * 42) matmul collapses to a
[512, 42] projection plus a pairwise broadcast-add, implemented on the PE
as a single K=43 matmul per output chunk: lhsT = [g'_i rows; ones row],
rhs = [periodic identity rows; g'_j flattened row].

Sharding: 8 cores = 4 i-blocks (128 rows) x 2 j-halves (256 cols).
Each core runs an identical (SPMD) program on a permuted 384-token slice:
cols 0:128 = its i-block tokens, cols 128:384 = its j-half tokens.

Scheduling: the PE runs at the 1.2 GHz mid pstate, so the kernel is
PE-cycle-bound between the input DMA and the output DMA. The program
keeps the PE stream dense with useful work only: gates -> mlp_in for the
two j-column halves (each followed immediately by its mlp_out projection
and flatten DMA) -> mlp_in for the i-columns + el during the flatten DMA
flight -> pairwise chunks. Output is written bf16 (host upcasts) to halve
the output DMA.
"""

import os
import sys

import numpy as np

for _p in ("/opt/trn_rl_repo", "/root/.axon_site/_ro/trn_rl_repo"):
    if os.path.isdir(_p) and _p not in sys.path:
        sys.path.insert(0, _p)

import ml_dtypes  # noqa: E402

import concourse.bacc as bacc  # noqa: E402
import concourse.bass as bass  # noqa: E402
import concourse.mybir as mybir  # noqa: E402
from concourse.bass_utils import run_bass_kernel_spmd  # noqa: E402
from concourse.tile import TileContext  # noqa: E402

BF16 = mybir.dt.bfloat16
F32 = mybir.dt.float32
AF = mybir.ActivationFunctionType

SEQ = 512
D_VEC = 425  # 100 + 25 + 300
NREL = 42
T = 384  # per-core tokens: 128 (i-block) + 256 (j-half)
NFLAT = 256 * NREL  # 10752 = per-core output row length
N_CHUNK = 512
N_CHUNKS = NFLAT // N_CHUNK  # 21
GRP = 4  # pairwise chunks fused per PSUM group / tanh / DMA
IC_PER = 16 * NREL  # 672: replication period for the identity pattern

# K-dim tiling of the 425-dim feature axis
KS = [(0, 128), (128, 256), (256, 384), (384, 425)]
# gate order in the stacked [425, 600] gate weight: i_f g_f o_f i_b g_b o_b
GATE_FUNCS = [AF.Sigmoid, AF.Tanh, AF.Sigmoid] * 2

# column groups of the per-core token slice for mlp_in: the two j-halves
# first (so their g' projections + flatten DMAs launch as early as
# possible), the i-block last (its matmuls cover the flatten DMA flight).
# (group, col_a, col_b, j_half_or_None)
COLGROUPS = [(1, 128, 256, 0), (1, 256, 384, 1), (0, 0, 128, None)]

# ---- packed bf16 constant layout: [128, NPK] ----
_SEGS = []  # name -> (rows, col_off, width)


def _seg(name, rows, width):
    off = _SEGS[-1][2] + _SEGS[-1][3] if _SEGS else 0
    _SEGS.append((name, rows, off, width))


# interleaved (vt_k, g6_k) pairs so the first gate matmuls can start as
# soon as the first small DMA lands. Then group-1 (j-half) mlp_in
# weights, then group-0 (i-block) weights, then mlp_out weight + the
# identity pattern.
for _k, (_a, _b) in enumerate(KS):
    _seg(f"vt{_k}", _b - _a, 384)
    _seg(f"g6{_k}", _b - _a, 600)
for _g in (1, 0):
    for _a2 in range(2):
        _seg(f"wh{_g}{_a2}", 100, 400)
    for _k, (_a, _b) in enumerate(KS):
        _seg(f"wv{_g}{_k}", _b - _a, 400)
_seg("wo", 101, 4 * NREL)
_seg("ic", NREL, IC_PER)
SEG = {s[0]: s for s in _SEGS}
NPK = _SEGS[-1][2] + _SEGS[-1][3]
# input DMA split points: one per (vt_k, g6_k) pair, then the g1 weight
# block, then the remainder (g0 weights + wo + ic)
PK_CUTS = [SEG[f"g6{_k}"][2] + SEG[f"g6{_k}"][3] for _k in range(4)]
PK_CUTS.append(SEG["wh00"][2])  # end of the g1 (j-half) weight block
PK_CUTS.append(NPK)


def _build_program():
    nc = bacc.Bacc()

    pk_d = nc.dram_tensor("pk", [128, NPK], BF16, kind="ExternalInput")
    bias_d = nc.dram_tensor("bias", [100, 11], F32, kind="ExternalInput")
    out_d = nc.dram_tensor("out", [128, NFLAT], BF16, kind="ExternalOutput")

    with TileContext(nc) as tc:
        with (
            tc.tile_pool(name="const", bufs=1) as cp,
            tc.tile_pool(name="work", bufs=3) as wp,
            tc.tile_pool(name="outp", bufs=3) as op_,
        ):
            # -------- early on-chip init (no DMA deps) --------
            # lhsT of the pairwise matmul: rows 0:42 = g'_i, row 42 = 1.0.
            # DVE partition base must be 32-aligned, so memset 32:43 and let
            # the later g' write overwrite rows 32:42.
            el = cp.tile([NREL + 1, 128], BF16, tag="el")
            nc.vector.memset(el[32 : NREL + 1, :], 1.0)
            # warmup activations absorb the two ACT table-set loads early
            # (they overlap the input DMA flight)
            warmsrc = cp.tile([1, 8], BF16, tag="warmsrc")
            nc.gpsimd.memset(warmsrc, 0.0)
            warm2 = cp.tile([1, 8], F32, tag="warm2")
            nc.scalar.activation(out=warm2, in_=warmsrc, func=AF.Sigmoid)
            nc.scalar.activation(out=warm2, in_=warmsrc, func=AF.Tanh)

            # -------- input DMAs (paired packed chunks + bias) --------
            # bias rides the scalar engine's HWDGE queue so it neither
            # waits behind nor delays the pk stream on sync.
            bias = cp.tile([100, 11], F32, tag="bias")
            nc.scalar.dma_start(out=bias, in_=bias_d[:, :])
            pk = cp.tile([128, NPK], BF16, tag="pk")
            prev = 0
            for cut in PK_CUTS:
                nc.sync.dma_start(out=pk[:, prev:cut], in_=pk_d[:, prev:cut])
                prev = cut

            def seg(name):
                _, rows, off, width = SEG[name]
                return pk[0:rows, off : off + width]

            vt = [seg(f"vt{k}") for k in range(4)]
            g6 = [seg(f"g6{k}") for k in range(4)]
            wh = [[seg(f"wh{g}{a}") for a in range(2)] for g in range(2)]
            wv = [[seg(f"wv{g}{k}") for k in range(4)] for g in range(2)]
            wo = seg("wo")
            ic = seg("ic")

            # pairwise rhs: rows 0:42 = periodic identity, row 42 = g'_j flat
            rr = cp.tile([NREL + 1, NFLAT], BF16, tag="rr")
            ic_rep = bass.AP(
                tensor=ic.tensor,
                offset=ic.offset,
                ap=[ic.ap[0], [0, NFLAT // IC_PER], ic.ap[1]],
            )
            nc.sync.dma_start(out=rr[0:NREL, :], in_=ic_rep)

            with tc.tile_pool(name="psum_pre", bufs=1, space="PSUM") as pp:
                # -------- LSTM gates (both dirs, f-gate skipped) --------
                # Per-direction ordering: i, g (then c = sig(i)*tanh(g) and
                # tanh(c) start immediately), then o, then h — shortens the
                # serial ACT chain to each direction's h.
                def gate(m):
                    pg = pp.tile([100, T], F32, tag="pg", bufs=3, name=f"pg{m}")
                    for k in range(4):
                        nc.tensor.matmul(
                            pg,
                            lhsT=g6[k][:, m * 100 : (m + 1) * 100],
                            rhs=vt[k],
                            start=(k == 0),
                            stop=(k == 3),
                        )
                    a_ = wp.tile([100, T], BF16, tag=f"act{m}", name=f"act{m}")
                    nc.scalar.activation(
                        out=a_,
                        in_=pg,
                        func=GATE_FUNCS[m],
                        bias=bias[0:100, m : m + 1],
                        scale=1.0,
                    )
                    return a_

                hh = []
                for d in range(2):
                    si = gate(3 * d)
                    tg = gate(3 * d + 1)
                    c_ = wp.tile([100, T], BF16, tag=f"c{d}")
                    nc.vector.tensor_mul(c_, si, tg)
                    tc_ = wp.tile([100, T], BF16, tag=f"tc{d}")
                    nc.scalar.activation(out=tc_, in_=c_, func=AF.Tanh)
                    so = gate(3 * d + 2)
                    h_ = cp.tile([100, T], BF16, tag=f"h{d}")
                    nc.vector.tensor_mul(h_, so, tc_)
                    hh.append(h_)

                # -------- mlp_in: fT [400, 384], one column group at a
                # time. Per group the first three m-slices' vec matmuls
                # (no h dependency) are issued before any h matmul, so the
                # PE keeps running while the LSTM ACT chain finishes.
                fm = []
                for m in range(4):
                    rows = 101 if m == 3 else 100
                    f_ = cp.tile([rows, T], BF16, tag=f"f{m}")
                    # fm[3] carries an extra all-ones row 100 so the
                    # natural-layout mlp_out can fold +b_out/2 in as a
                    # rank-1 term (wo row 100 holds b_out/2). Memset base
                    # must be 32-aligned: set 96:101, rows 96:100 are
                    # overwritten by the bias adds below.
                    if m == 3:
                        nc.vector.memset(f_[96:101, :], 1.0)
                    fm.append(f_)

                for cgi, (g, ca, cb, jh) in enumerate(COLGROUPS):
                    pfs = [None] * 4

                    def vec_part(m, g=g, ca=ca, cb=cb, cgi=cgi, pfs=pfs):
                        pf = pp.tile(
                            [100, cb - ca], F32, tag="pf", bufs=3, name=f"pf{cgi}_{m}"
                        )
                        pfs[m] = pf
                        ms = slice(m * 100, (m + 1) * 100)
                        for k in range(4):
                            nc.tensor.matmul(
                                pf,
                                lhsT=wv[g][k][:, ms],
                                rhs=vt[k][:, ca:cb],
                                start=(k == 0),
                                stop=False,
                            )

                    def h_part(m, g=g, ca=ca, cb=cb, pfs=pfs):
                        ms = slice(m * 100, (m + 1) * 100)
                        for a in range(2):
                            nc.tensor.matmul(
                                pfs[m],
                                lhsT=wh[g][a][:, ms],
                                rhs=hh[a][:, ca:cb],
                                start=False,
                                stop=(a == 1),
                            )
                        nc.vector.tensor_scalar_add(
                            fm[m][0:100, ca:cb],
                            pfs[m],
                            bias[0:100, 6 + m : 7 + m],
                        )

                    vec_part(0)
                    vec_part(1)
                    vec_part(2)
                    h_part(0)
                    vec_part(3)
                    h_part(1)
                    h_part(2)
                    h_part(3)

                    if jh is not None:
                        # mlp_out for this j-half in natural layout
                        # (g'_nat[t, r]; fT is already [f, t] so no
                        # transposes), then flatten into rr row 42 —
                        # the DMA flight is covered by the next column
                        # group's matmuls.
                        png = pp.tile(
                            [128, NREL], F32, tag="pq", bufs=2, name=f"png{jh}"
                        )
                        for m in range(4):
                            kr = 101 if m == 3 else 100
                            nc.tensor.matmul(
                                png,
                                lhsT=fm[m][0:kr, ca:cb],
                                rhs=wo[0:kr, m * NREL : (m + 1) * NREL],
                                start=(m == 0),
                                stop=(m == 3),
                            )
                        tj = wp.tile([128, NREL], BF16, tag="tj", name=f"tj{jh}")
                        nc.vector.tensor_copy(tj, png)
                        nc.sync.dma_start(
                            out=rr[
                                NREL : NREL + 1,
                                jh * 128 * NREL : (jh + 1) * 128 * NREL,
                            ],
                            in_=tj,
                        )
                    else:
                        # mlp_out for the i-block: g'T [42, 128] (+ b/2)
                        pl = pp.tile([NREL, 128], F32, tag="pq", bufs=2, name="pl")
                        for m in range(4):
                            nc.tensor.matmul(
                                pl,
                                lhsT=wo[0:100, m * NREL : (m + 1) * NREL],
                                rhs=fm[m][0:100, 0:128],
                                start=(m == 0),
                                stop=(m == 3),
                            )
                        nc.vector.tensor_scalar_add(
                            el[0:NREL, :], pl, bias[0:NREL, 10:11]
                        )

            # -------- pairwise: tanh(g'_i + g'_j) --------
            # Small first group lets the (pacing) ACT tanh stream start
            # early; the tanh output is bf16 so the output DMA is half
            # the bytes of fp32.
            grp_plan = (1, 4, 4, 4, 4, 4)
            with tc.tile_pool(name="psum_pair", bufs=2, space="PSUM") as pq:
                c = 0
                for nch in grp_plan:
                    ppair = pq.tile([128, GRP * N_CHUNK], F32, tag="ppair")
                    base = c * N_CHUNK
                    for q in range(nch):
                        nc.tensor.matmul(
                            ppair[:, q * N_CHUNK : (q + 1) * N_CHUNK],
                            lhsT=el,
                            rhs=rr[:, (c + q) * N_CHUNK : (c + q + 1) * N_CHUNK],
                            start=True,
                            stop=True,
                        )
                    ot = op_.tile([128, GRP * N_CHUNK], BF16, tag="ot")
                    nc.scalar.activation(
                        out=ot[:, 0 : nch * N_CHUNK],
                        in_=ppair[:, 0 : nch * N_CHUNK],
                        func=AF.Tanh,
                    )
                    nc.sync.dma_start(
                        out=out_d[:, base : base + nch * N_CHUNK],
                        in_=ot[:, 0 : nch * N_CHUNK],
                    )
                    c += nch

    nc.finalize()
    return nc


def _host_prepare(inputs):
    """Gather embeddings + lay out weights; returns per-core in_maps."""
    bf = ml_dtypes.bfloat16
    wi = np.asarray(inputs["word_idx"]).astype(np.int64)
    pi = np.asarray(inputs["pos_idx"]).astype(np.int64)
    ei = np.asarray(inputs["ext_idx"]).astype(np.int64)
    we = np.asarray(inputs["word_emb"], np.float32)
    pe = np.asarray(inputs["pos_emb"], np.float32)
    xe = np.asarray(inputs["ext_emb"], np.float32)
    vec = np.concatenate([we[wi], pe[pi], xe[ei]], axis=-1)  # [512, 425] f32

    w_ih_f = np.asarray(inputs["w_ih_f"], np.float32)
    w_ih_b = np.asarray(inputs["w_ih_b"], np.float32)
    b_f = np.asarray(inputs["b_f"], np.float32)
    b_b = np.asarray(inputs["b_b"], np.float32)
    w_mlp_in = np.asarray(inputs["w_mlp_in"], np.float32)
    b_mlp_in = np.asarray(inputs["b_mlp_in"], np.float32)
    w_mlp_out = np.asarray(inputs["w_mlp_out"], np.float32)
    b_mlp_out = np.asarray(inputs["b_mlp_out"], np.float32)

    # stacked gate weights [425, 600]: i_f g_f o_f i_b g_b o_b (f unused)
    w6 = np.concatenate(
        [
            w_ih_f[0:100],
            w_ih_f[200:300],
            w_ih_f[300:400],
            w_ih_b[0:100],
            w_ih_b[200:300],
            w_ih_b[300:400],
        ],
        axis=0,
    ).T  # [425, 600]

    bias = np.zeros((100, 11), np.float32)
    for m, sl in enumerate(
        [b_f[0:100], b_f[200:300], b_f[300:400], b_b[0:100], b_b[200:300], b_b[300:400]]
    ):
        bias[:, m] = sl
    bias[:, 6:10] = b_mlp_in.reshape(4, 100).T
    bias[0:NREL, 10] = 0.5 * b_mlp_out

    # row 100: b_out/2 for the natural-layout mlp_out rank-1 bias fold
    # (only the m=3 block's slice is ever read at K=101)
    wo = np.zeros((101, 4 * NREL), np.float32)
    wout_t = w_mlp_out.T  # [400, 42]
    for m in range(4):
        wo[0:100, m * NREL : (m + 1) * NREL] = wout_t[m * 100 : (m + 1) * 100]
        wo[100, m * NREL : (m + 1) * NREL] = 0.5 * b_mlp_out

    # periodic identity block for the pairwise broadcast matmul
    ic = np.zeros((NREL, IC_PER), np.float32)
    cols = np.arange(IC_PER)
    ic[cols % NREL, cols] = 1.0

    def halves(hv):
        if hv:  # cat = [h, vec]
            whx = w_mlp_in[:, 0:200].T  # [200, 400] rows = h features
            wvx = w_mlp_in[:, 200:625].T  # [425, 400] rows = vec features
        else:  # cat = [vec, h]
            whx = w_mlp_in[:, 425:625].T
            wvx = w_mlp_in[:, 0:425].T
        return whx, wvx

    def fill(pk, name, arr):
        _, rows, off, width = SEG[name]
        assert arr.shape == (rows, width), (name, arr.shape, rows, width)
        pk[0:rows, off : off + width] = arr

    in_maps = []
    for core in range(8):
        ib, jh = core // 2, core % 2
        toks = np.concatenate(
            [np.arange(ib * 128, (ib + 1) * 128), np.arange(jh * 256, (jh + 1) * 256)]
        )
        vect = vec[toks].T  # [425, 384]
        g0h, g0v = halves(ib < 2)
        g1h, g1v = halves(jh == 0)

        pk = np.zeros((128, NPK), np.float32)
        for k, (a, b) in enumerate(KS):
            fill(pk, f"vt{k}", vect[a:b])
            fill(pk, f"g6{k}", w6[a:b])
        for g, (gh, gv) in enumerate([(g0h, g0v), (g1h, g1v)]):
            for a in range(2):
                fill(pk, f"wh{g}{a}", gh[a * 100 : (a + 1) * 100])
            for k, (a, b) in enumerate(KS):
                fill(pk, f"wv{g}{k}", gv[a:b])
        fill(pk, "wo", wo)
        fill(pk, "ic", ic)
        in_maps.append(dict(pk=pk.astype(bf), bias=bias))
    return in_maps


_CACHED_NC = None


def kernel(**inputs):
    global _CACHED_NC
    in_maps = _host_prepare(inputs)
    if _CACHED_NC is None:
        _CACHED_NC = _build_program()
    res = run_bass_kernel_spmd(_CACHED_NC, in_maps, list(range(8)))
    full = np.empty((SEQ, SEQ, NREL), np.float32)
    for core in range(8):
        ib, jh = core // 2, core % 2
        blk = res.results[core]["out"].astype(np.float32).reshape(128, 256, NREL)
        full[ib * 128 : (ib + 1) * 128, jh * 256 : (jh + 1) * 256, :] = blk
    return full


if __name__ == "__main__":
    rng = np.random.default_rng(0)
    demo = dict(
        word_idx=rng.integers(0, 50000, 512),
        pos_idx=rng.integers(0, 48, 512),
        ext_idx=rng.integers(0, 100000, 512),
        word_emb=rng.standard_normal((50000, 100), np.float32) * 0.05,
        pos_emb=rng.standard_normal((48, 25), np.float32) * 0.05,
        ext_emb=rng.standard_normal((100000, 300), np.float32) * 0.05,
        w_ih_f=rng.standard_normal((400, 425), np.float32) * 0.05,
        b_f=rng.standard_normal(400).astype(np.float32) * 0.05,
        w_ih_b=rng.standard_normal((400, 425), np.float32) * 0.05,
        b_b=rng.standard_normal(400).astype(np.float32) * 0.05,
        w_mlp_in=rng.standard_normal((400, 625), np.float32) * 0.05,
        b_mlp_in=rng.standard_normal(400).astype(np.float32) * 0.05,
        w_mlp_out=rng.standard_normal((42, 400), np.float32) * 0.05,
        b_mlp_out=rng.standard_normal(42).astype(np.float32) * 0.05,
    )
    out = kernel(**demo)
    print("out", out.shape, out.dtype, float(np.abs(out).max()))


# revision 11
# speedup vs baseline: 1.2774x; 1.1054x over previous
"""Trainium2 Bass kernel for nn_BiLSTM pairwise-scores problem.

Math (reference):
  vec  = concat(word_emb[wi], pos_emb[pi], ext_emb[ei])          [512, 425]
  h    = concat(lstm_cell_f(vec), lstm_cell_b(vec))              [512, 200]
  cat  = [h, vec] for t <= 255 else [vec, h]                     [512, 625]
  f    = cat @ w_mlp_in.T + b_mlp_in                             [512, 400]
  out  = tanh((f[:,None,:] + f[None,:,:]) @ w_mlp_out.T + b_out) [512, 512, 42]

Key factorization: (f_i + f_j) @ W.T + b = g'_i + g'_j with
g' = f @ W.T + b/2, so the O(n^2 * 400 * 42) matmul collapses to a
[512, 42] projection plus a pairwise broadcast-add, implemented on the PE
as a single K=43 matmul per output chunk: lhsT = [g'_i rows; ones row],
rhs = [periodic identity rows; g'_j flattened row].

Sharding: 8 cores = 4 i-blocks (128 rows) x 2 j-halves (256 cols).
Each core runs an identical (SPMD) program on a permuted 384-token slice:
cols 0:128 = its i-block tokens, cols 128:384 = its j-half tokens.

Scheduling: the PE runs at the 1.2 GHz mid pstate, so the kernel is
PE-cycle-bound between the input DMA and the output DMA. The program
keeps the PE stream dense with useful work only: gates -> mlp_in for the
two j-column halves (each followed immediately by its mlp_out projection
and flatten DMA) -> mlp_in for the i-columns + el during the flatten DMA
flight -> pairwise chunks. Output is written bf16 (host upcasts) to halve
the output DMA.
"""

import os
import sys

import numpy as np

for _p in ("/opt/trn_rl_repo", "/root/.axon_site/_ro/trn_rl_repo"):
    if os.path.isdir(_p) and _p not in sys.path:
        sys.path.insert(0, _p)

import ml_dtypes  # noqa: E402

import concourse.bacc as bacc  # noqa: E402
import concourse.bass as bass  # noqa: E402
import concourse.mybir as mybir  # noqa: E402
from concourse.bass_utils import run_bass_kernel_spmd  # noqa: E402
from concourse.tile import TileContext  # noqa: E402

BF16 = mybir.dt.bfloat16
F32 = mybir.dt.float32
AF = mybir.ActivationFunctionType

SEQ = 512
D_VEC = 425  # 100 + 25 + 300
NREL = 42
T = 384  # per-core tokens: 128 (i-block) + 256 (j-half)
NFLAT = 256 * NREL  # 10752 = per-core output row length
N_CHUNK = 512
N_CHUNKS = NFLAT // N_CHUNK  # 21
GRP = 4  # pairwise chunks fused per PSUM group / tanh / DMA
IC_PER = 16 * NREL  # 672: replication period for the identity pattern

# K-dim tiling of the 425-dim feature axis. Near-even tiles (107/106/
# 106/106) rather than 128/128/128/41: a <=64-row tile makes the PE drop
# into half-array row-group mode and the mode switch costs ~150-300ns
# per transition, which dwarfs the saved rows.
KS = [(0, 107), (107, 213), (213, 319), (319, 425)]
# gate order in the stacked [425, 600] gate weight: i_f g_f o_f i_b g_b o_b
GATE_FUNCS = [AF.Sigmoid, AF.Tanh, AF.Sigmoid] * 2

# column groups of the per-core token slice for mlp_in: the two j-halves
# first (so their g' projections + flatten DMAs launch as early as
# possible), the i-block last (its matmuls cover the flatten DMA flight).
# (group, col_a, col_b, j_half_or_None)
COLGROUPS = [(1, 128, 256, 0), (1, 256, 384, 1), (0, 0, 128, None)]

# ---- packed bf16 constant layout: [128, NPK] ----
_SEGS = []  # name -> (rows, col_off, width)


def _seg(name, rows, width):
    off = _SEGS[-1][2] + _SEGS[-1][3] if _SEGS else 0
    _SEGS.append((name, rows, off, width))


# interleaved (vt_k, g6_k) pairs so the first gate matmuls can start as
# soon as the first small DMA lands. Then group-1 (j-half) mlp_in
# weights, then group-0 (i-block) weights, then mlp_out weight + the
# identity pattern.
for _k, (_a, _b) in enumerate(KS):
    _seg(f"vt{_k}", _b - _a, 384)
    _seg(f"g6{_k}", _b - _a, 600)
for _g in (1, 0):
    for _a2 in range(2):
        _seg(f"wh{_g}{_a2}", 100, 400)
    for _k, (_a, _b) in enumerate(KS):
        _seg(f"wv{_g}{_k}", _b - _a, 400)
_seg("wo", 101, 4 * NREL)
_seg("ic", NREL, IC_PER)
SEG = {s[0]: s for s in _SEGS}
NPK = _SEGS[-1][2] + _SEGS[-1][3]
# input DMA split points: one per (vt_k, g6_k) pair, then the g1 weight
# block, then the remainder (g0 weights + wo + ic)
PK_CUTS = [SEG[f"g6{_k}"][2] + SEG[f"g6{_k}"][3] for _k in range(4)]
PK_CUTS.append(SEG["wh00"][2])  # end of the g1 (j-half) weight block
PK_CUTS.append(NPK)


def _build_program():
    nc = bacc.Bacc()

    pk_d = nc.dram_tensor("pk", [128, NPK], BF16, kind="ExternalInput")
    bias_d = nc.dram_tensor("bias", [100, 11], F32, kind="ExternalInput")
    out_d = nc.dram_tensor("out", [128, NFLAT], BF16, kind="ExternalOutput")

    with TileContext(nc) as tc:
        with (
            tc.tile_pool(name="const", bufs=1) as cp,
            tc.tile_pool(name="work", bufs=3) as wp,
            tc.tile_pool(name="outp", bufs=3) as op_,
        ):
            # -------- early on-chip init (no DMA deps) --------
            # lhsT of the pairwise matmul: rows 0:42 = g'_i, row 42 = 1.0.
            # DVE partition base must be 32-aligned, so memset 32:43 and let
            # the later g' write overwrite rows 32:42.
            el = cp.tile([NREL + 1, 128], BF16, tag="el")
            nc.vector.memset(el[32 : NREL + 1, :], 1.0)
            # warmup activations absorb the two ACT table-set loads early
            # (they overlap the input DMA flight)
            warmsrc = cp.tile([1, 8], BF16, tag="warmsrc")
            nc.gpsimd.memset(warmsrc, 0.0)
            warm2 = cp.tile([1, 8], F32, tag="warm2")
            nc.scalar.activation(out=warm2, in_=warmsrc, func=AF.Sigmoid)
            nc.scalar.activation(out=warm2, in_=warmsrc, func=AF.Tanh)

            # -------- input DMAs (paired packed chunks + bias) --------
            # bias rides the scalar engine's HWDGE queue so it neither
            # waits behind nor delays the pk stream on sync.
            bias = cp.tile([100, 11], F32, tag="bias")
            nc.scalar.dma_start(out=bias, in_=bias_d[:, :])
            pk = cp.tile([128, NPK], BF16, tag="pk")
            prev = 0
            for cut in PK_CUTS:
                nc.sync.dma_start(out=pk[:, prev:cut], in_=pk_d[:, prev:cut])
                prev = cut

            def seg(name):
                _, rows, off, width = SEG[name]
                return pk[0:rows, off : off + width]

            vt = [seg(f"vt{k}") for k in range(4)]
            g6 = [seg(f"g6{k}") for k in range(4)]
            wh = [[seg(f"wh{g}{a}") for a in range(2)] for g in range(2)]
            wv = [[seg(f"wv{g}{k}") for k in range(4)] for g in range(2)]
            wo = seg("wo")
            ic = seg("ic")

            # pairwise rhs: rows 0:42 = periodic identity, row 42 = g'_j flat
            rr = cp.tile([NREL + 1, NFLAT], BF16, tag="rr")
            ic_rep = bass.AP(
                tensor=ic.tensor,
                offset=ic.offset,
                ap=[ic.ap[0], [0, NFLAT // IC_PER], ic.ap[1]],
            )
            nc.sync.dma_start(out=rr[0:NREL, :], in_=ic_rep)

            with tc.tile_pool(name="psum_pre", bufs=1, space="PSUM") as pp:
                # -------- LSTM gates (both dirs, f-gate skipped) --------
                # Per-direction ordering: i, g (then c = sig(i)*tanh(g) and
                # tanh(c) start immediately), then o, then h — shortens the
                # serial ACT chain to each direction's h.
                def gate(m):
                    pg = pp.tile([100, T], F32, tag="pg", bufs=4, name=f"pg{m}")
                    for k in range(4):
                        nc.tensor.matmul(
                            pg,
                            lhsT=g6[k][:, m * 100 : (m + 1) * 100],
                            rhs=vt[k],
                            start=(k == 0),
                            stop=(k == 3),
                        )
                    a_ = wp.tile([100, T], BF16, tag=f"act{m}", name=f"act{m}")
                    nc.scalar.activation(
                        out=a_,
                        in_=pg,
                        func=GATE_FUNCS[m],
                        bias=bias[0:100, m : m + 1],
                        scale=1.0,
                    )
                    return a_

                hh = []
                for d in range(2):
                    si = gate(3 * d)
                    tg = gate(3 * d + 1)
                    c_ = wp.tile([100, T], BF16, tag=f"c{d}")
                    nc.vector.tensor_mul(c_, si, tg)
                    tc_ = wp.tile([100, T], BF16, tag=f"tc{d}")
                    nc.scalar.activation(out=tc_, in_=c_, func=AF.Tanh)
                    so = gate(3 * d + 2)
                    h_ = cp.tile([100, T], BF16, tag=f"h{d}")
                    nc.vector.tensor_mul(h_, so, tc_)
                    hh.append(h_)

                # -------- mlp_in: fT [400, 384], one column group at a
                # time. Per group the first three m-slices' vec matmuls
                # (no h dependency) are issued before any h matmul, so the
                # PE keeps running while the LSTM ACT chain finishes.
                fm = []
                for m in range(4):
                    rows = 101 if m == 3 else 100
                    f_ = cp.tile([rows, T], BF16, tag=f"f{m}")
                    # fm[3] carries an extra all-ones row 100 so the
                    # natural-layout mlp_out can fold +b_out/2 in as a
                    # rank-1 term (wo row 100 holds b_out/2). Memset base
                    # must be 32-aligned: set 96:101, rows 96:100 are
                    # overwritten by the bias adds below.
                    if m == 3:
                        nc.vector.memset(f_[96:101, :], 1.0)
                    fm.append(f_)

                for cgi, (g, ca, cb, jh) in enumerate(COLGROUPS):
                    pfs = [None] * 4

                    def vec_part(m, g=g, ca=ca, cb=cb, cgi=cgi, pfs=pfs):
                        pf = pp.tile(
                            [100, cb - ca], F32, tag="pf", bufs=3, name=f"pf{cgi}_{m}"
                        )
                        pfs[m] = pf
                        ms = slice(m * 100, (m + 1) * 100)
                        for k in range(4):
                            nc.tensor.matmul(
                                pf,
                                lhsT=wv[g][k][:, ms],
                                rhs=vt[k][:, ca:cb],
                                start=(k == 0),
                                stop=False,
                            )

                    def h_part(m, g=g, ca=ca, cb=cb, pfs=pfs):
                        ms = slice(m * 100, (m + 1) * 100)
                        for a in range(2):
                            nc.tensor.matmul(
                                pfs[m],
                                lhsT=wh[g][a][:, ms],
                                rhs=hh[a][:, ca:cb],
                                start=False,
                                stop=(a == 1),
                            )
                        nc.vector.tensor_scalar_add(
                            fm[m][0:100, ca:cb],
                            pfs[m],
                            bias[0:100, 6 + m : 7 + m],
                        )

                    vec_part(0)
                    vec_part(1)
                    vec_part(2)
                    h_part(0)
                    vec_part(3)
                    h_part(1)
                    h_part(2)
                    h_part(3)

                    if jh is not None:
                        # mlp_out for this j-half in natural layout
                        # (g'_nat[t, r]; fT is already [f, t] so no
                        # transposes), then flatten into rr row 42 —
                        # the DMA flight is covered by the next column
                        # group's matmuls.
                        png = pp.tile(
                            [128, NREL], F32, tag="pq", bufs=1, name=f"png{jh}"
                        )
                        for m in range(4):
                            kr = 101 if m == 3 else 100
                            nc.tensor.matmul(
                                png,
                                lhsT=fm[m][0:kr, ca:cb],
                                rhs=wo[0:kr, m * NREL : (m + 1) * NREL],
                                start=(m == 0),
                                stop=(m == 3),
                            )
                        tj = wp.tile([128, NREL], BF16, tag="tj", name=f"tj{jh}")
                        nc.vector.tensor_copy(tj, png)
                        nc.sync.dma_start(
                            out=rr[
                                NREL : NREL + 1,
                                jh * 128 * NREL : (jh + 1) * 128 * NREL,
                            ],
                            in_=tj,
                        )
                    else:
                        # mlp_out for the i-block: g'T [42, 128] (+ b/2)
                        pl = pp.tile([NREL, 128], F32, tag="pq", bufs=1, name="pl")
                        for m in range(4):
                            nc.tensor.matmul(
                                pl,
                                lhsT=wo[0:100, m * NREL : (m + 1) * NREL],
                                rhs=fm[m][0:100, 0:128],
                                start=(m == 0),
                                stop=(m == 3),
                            )
                        nc.vector.tensor_scalar_add(
                            el[0:NREL, :], pl, bias[0:NREL, 10:11]
                        )

            # -------- pairwise: tanh(g'_i + g'_j) --------
            # Small first group lets the (pacing) ACT tanh stream start
            # early; the tanh output is bf16 so the output DMA is half
            # the bytes of fp32.
            grp_plan = (1, 4, 4, 4, 4, 4)
            with tc.tile_pool(name="psum_pair", bufs=2, space="PSUM") as pq:
                c = 0
                for nch in grp_plan:
                    ppair = pq.tile([128, GRP * N_CHUNK], F32, tag="ppair")
                    base = c * N_CHUNK
                    for q in range(nch):
                        nc.tensor.matmul(
                            ppair[:, q * N_CHUNK : (q + 1) * N_CHUNK],
                            lhsT=el,
                            rhs=rr[:, (c + q) * N_CHUNK : (c + q + 1) * N_CHUNK],
                            start=True,
                            stop=True,
                        )
                    ot = op_.tile([128, GRP * N_CHUNK], BF16, tag="ot")
                    nc.scalar.activation(
                        out=ot[:, 0 : nch * N_CHUNK],
                        in_=ppair[:, 0 : nch * N_CHUNK],
                        func=AF.Tanh,
                    )
                    # split each group's output across two DMA queues so
                    # the post-tanh drain runs at 2x queue bandwidth
                    half = (nch * N_CHUNK) // 2
                    nc.sync.dma_start(
                        out=out_d[:, base : base + half],
                        in_=ot[:, 0:half],
                    )
                    nc.gpsimd.dma_start(
                        out=out_d[:, base + half : base + nch * N_CHUNK],
                        in_=ot[:, half : nch * N_CHUNK],
                    )
                    c += nch

    nc.finalize()
    return nc


def _host_prepare(inputs):
    """Gather embeddings + lay out weights; returns per-core in_maps."""
    bf = ml_dtypes.bfloat16
    wi = np.asarray(inputs["word_idx"]).astype(np.int64)
    pi = np.asarray(inputs["pos_idx"]).astype(np.int64)
    ei = np.asarray(inputs["ext_idx"]).astype(np.int64)
    we = np.asarray(inputs["word_emb"], np.float32)
    pe = np.asarray(inputs["pos_emb"], np.float32)
    xe = np.asarray(inputs["ext_emb"], np.float32)
    vec = np.concatenate([we[wi], pe[pi], xe[ei]], axis=-1)  # [512, 425] f32

    w_ih_f = np.asarray(inputs["w_ih_f"], np.float32)
    w_ih_b = np.asarray(inputs["w_ih_b"], np.float32)
    b_f = np.asarray(inputs["b_f"], np.float32)
    b_b = np.asarray(inputs["b_b"], np.float32)
    w_mlp_in = np.asarray(inputs["w_mlp_in"], np.float32)
    b_mlp_in = np.asarray(inputs["b_mlp_in"], np.float32)
    w_mlp_out = np.asarray(inputs["w_mlp_out"], np.float32)
    b_mlp_out = np.asarray(inputs["b_mlp_out"], np.float32)

    # stacked gate weights [425, 600]: i_f g_f o_f i_b g_b o_b (f unused)
    w6 = np.concatenate(
        [
            w_ih_f[0:100],
            w_ih_f[200:300],
            w_ih_f[300:400],
            w_ih_b[0:100],
            w_ih_b[200:300],
            w_ih_b[300:400],
        ],
        axis=0,
    ).T  # [425, 600]

    bias = np.zeros((100, 11), np.float32)
    for m, sl in enumerate(
        [b_f[0:100], b_f[200:300], b_f[300:400], b_b[0:100], b_b[200:300], b_b[300:400]]
    ):
        bias[:, m] = sl
    bias[:, 6:10] = b_mlp_in.reshape(4, 100).T
    bias[0:NREL, 10] = 0.5 * b_mlp_out

    # row 100: b_out/2 for the natural-layout mlp_out rank-1 bias fold
    # (only the m=3 block's slice is ever read at K=101)
    wo = np.zeros((101, 4 * NREL), np.float32)
    wout_t = w_mlp_out.T  # [400, 42]
    for m in range(4):
        wo[0:100, m * NREL : (m + 1) * NREL] = wout_t[m * 100 : (m + 1) * 100]
        wo[100, m * NREL : (m + 1) * NREL] = 0.5 * b_mlp_out

    # periodic identity block for the pairwise broadcast matmul
    ic = np.zeros((NREL, IC_PER), np.float32)
    cols = np.arange(IC_PER)
    ic[cols % NREL, cols] = 1.0

    def halves(hv):
        if hv:  # cat = [h, vec]
            whx = w_mlp_in[:, 0:200].T  # [200, 400] rows = h features
            wvx = w_mlp_in[:, 200:625].T  # [425, 400] rows = vec features
        else:  # cat = [vec, h]
            whx = w_mlp_in[:, 425:625].T
            wvx = w_mlp_in[:, 0:425].T
        return whx, wvx

    def fill(pk, name, arr):
        _, rows, off, width = SEG[name]
        assert arr.shape == (rows, width), (name, arr.shape, rows, width)
        pk[0:rows, off : off + width] = arr

    in_maps = []
    for core in range(8):
        ib, jh = core // 2, core % 2
        toks = np.concatenate(
            [np.arange(ib * 128, (ib + 1) * 128), np.arange(jh * 256, (jh + 1) * 256)]
        )
        vect = vec[toks].T  # [425, 384]
        g0h, g0v = halves(ib < 2)
        g1h, g1v = halves(jh == 0)

        pk = np.zeros((128, NPK), np.float32)
        for k, (a, b) in enumerate(KS):
            fill(pk, f"vt{k}", vect[a:b])
            fill(pk, f"g6{k}", w6[a:b])
        for g, (gh, gv) in enumerate([(g0h, g0v), (g1h, g1v)]):
            for a in range(2):
                fill(pk, f"wh{g}{a}", gh[a * 100 : (a + 1) * 100])
            for k, (a, b) in enumerate(KS):
                fill(pk, f"wv{g}{k}", gv[a:b])
        fill(pk, "wo", wo)
        fill(pk, "ic", ic)
        in_maps.append(dict(pk=pk.astype(bf), bias=bias))
    return in_maps


_CACHED_NC = None


def kernel(**inputs):
    global _CACHED_NC
    in_maps = _host_prepare(inputs)
    if _CACHED_NC is None:
        _CACHED_NC = _build_program()
    res = run_bass_kernel_spmd(_CACHED_NC, in_maps, list(range(8)))
    full = np.empty((SEQ, SEQ, NREL), np.float32)
    for core in range(8):
        ib, jh = core // 2, core % 2
        blk = res.results[core]["out"].astype(np.float32).reshape(128, 256, NREL)
        full[ib * 128 : (ib + 1) * 128, jh * 256 : (jh + 1) * 256, :] = blk
    return full


if __name__ == "__main__":
    rng = np.random.default_rng(0)
    demo = dict(
        word_idx=rng.integers(0, 50000, 512),
        pos_idx=rng.integers(0, 48, 512),
        ext_idx=rng.integers(0, 100000, 512),
        word_emb=rng.standard_normal((50000, 100), np.float32) * 0.05,
        pos_emb=rng.standard_normal((48, 25), np.float32) * 0.05,
        ext_emb=rng.standard_normal((100000, 300), np.float32) * 0.05,
        w_ih_f=rng.standard_normal((400, 425), np.float32) * 0.05,
        b_f=rng.standard_normal(400).astype(np.float32) * 0.05,
        w_ih_b=rng.standard_normal((400, 425), np.float32) * 0.05,
        b_b=rng.standard_normal(400).astype(np.float32) * 0.05,
        w_mlp_in=rng.standard_normal((400, 625), np.float32) * 0.05,
        b_mlp_in=rng.standard_normal(400).astype(np.float32) * 0.05,
        w_mlp_out=rng.standard_normal((42, 400), np.float32) * 0.05,
        b_mlp_out=rng.standard_normal(42).astype(np.float32) * 0.05,
    )
    out = kernel(**demo)
    print("out", out.shape, out.dtype, float(np.abs(out).max()))
